# revision 4
# baseline (speedup 1.0000x reference)
"""Multi-head attention (B=2, T=2048, D=1024, H=16, Dh=64) on 8 TRN2 NeuronCores.

Sharding: core c = 4*b + g  ->  batch b in {0,1}, head-group g in {0..3}
(4 heads per core: data parallel on batch, tensor parallel on heads).
Each core computes, for its batch element and its 4 heads:

  Q.T/K.T = Wq/k_shard.T @ x.T + b      [256, 2048]  (head-dim on partitions)
  V'      = x @ Wv_interleaved + b      [2048, 260]  ([V_h | 1] per head)
  per head pair (2p, 2p+1), per 512-wide i-chunk:
    S.T   = K_h Q_h.T                   (two K=64 matmuls on disjoint PE
                                         row groups -> run concurrently)
    P.T   = exp(S.T / 8)                (no max-subtraction: |S|/8 <~ 6)
    acc   = [V_h | 1].T @ P.T           [65, 512]  row 64 = softmax denom
    attnT = acc[:64] * (1/acc[64])
  partial = attnT.T @ Wout_shard        [2048, 1024]  (f16 out, host-summed)

The partial sum over the 4 head groups plus b_out is done on the host
("all-reduce after out_proj"), as is the batch unshard.

The softmax exp stream is split across two engines so neither serializes
the kernel: the scalar engine (ACT) computes true exp for most j-tiles;
for j in DVE_JS the vector engine computes a bit-trick exp2 -- one
tensor_scalar (x*a+b -> int16 bit pattern read as f16) plus one custom
DVE op that polishes the mantissa-linear error with a quadratic in
m = 1+frac (recovered exactly via bitwise and/or), max rel err ~5e-3.
PSUM evictions (V tiles, out-proj tiles, denominator rows) run on the
scalar engine; softmax normalization runs on the vector engine with the
reciprocal row broadcast across partitions by a 0-stride DMA.
"""

import os
import numpy as np

B, T, D = 2, 2048, 1024
H, DH = 16, 64
NCORES, GROUPS = 8, 4
HPC = H // GROUPS        # 4 heads per core
F = HPC * DH             # 256 features per core
FT = F // 128            # 2 feature tiles / head pairs
KTN = D // 128           # 8 contraction tiles
TT = T // 128            # 16 token tiles
NCH = 512                # matmul free-dim chunk
VW = DH + 1              # 65: V plus ones column
VF = HPC * VW            # 260: interleaved [V_h | 1] x 4 heads

# exp split: j-tiles per group computed on the DVE instead of ACT
DVE_JS = frozenset({2, 5, 8, 11, 14})

# f16 Schraudolph constants: i16 bits = round(s*ALPHA + BETA), then
# polish out = pe0 * ((m + PB) * m + PC), m = or(and(bits, MASK), 1.0).
# BETA includes the polish normalization (c0_adj = 2.0496...).
EXP_ALPHA = 184.6649652337873
EXP_BETA = 13261.182453842255
EXP_PB = -2.960537740957013
EXP_PC = 6.080904660347446
EXP_MASK = 0x007FE000

_prog = None
LAST_RESULT = None


def _register_exp_polish():
    """Register the EXP_POLISH_ANT custom DVE op (quadratic mantissa polish)."""
    import concourse.dve_ops as dom

    name = "EXP_POLISH_ANT"
    for op in dom.OPS:
        if op.name == name:
            return op
    from concourse.dve_spec import (
        Spec, Src0, C0, C1, C2, C3, Bin, AluOp, _spill_c3_to_src1, _has_src1, lower,
    )
    from concourse.dve_uop import DveOpSpec

    m = Bin(AluOp.BITWISE_OR, Bin(AluOp.BITWISE_AND, Src0, C0), C1)
    body = _spill_c3_to_src1(Src0 * ((m + C2) * m + C3))

    def ref(in0, in1, s0, s1, imm2):
        x = np.asarray(in0, np.float32)
        mask = np.asarray(s0, np.float32).view(np.uint32)
        orv = np.asarray(s1, np.float32).view(np.uint32)
        mm = ((x.view(np.uint32) & mask) | orv).view(np.float32)
        c = np.asarray(in1, np.float32)
        return (x * ((mm + np.float32(imm2)) * mm + c)).astype(np.float32)

    spec = Spec(body=body, reference=ref)
    row = max(dom._SUB_OPCODE_FOR_NAME.values()) + 1
    assert row < 0x20, "no free custom-DVE opcode row"
    dom._SUB_OPCODE_FOR_NAME[name] = row
    shas = {}
    for ver in ("v3", "v4"):
        try:
            shas[ver] = DveOpSpec(
                name=name, opcode=row, uops=lower(spec, ver=ver),
                rd1_en=_has_src1(spec),
            ).sha(ver)
        except Exception:
            if ver == "v3":
                raise  # TRN2 is v3; v4 is best-effort
    op = dom.DveOp(name, spec, subdim=False, uops_sha=shas)
    dom.OPS.append(op)
    dom.CUSTOM_DVE_SPECS[name] = spec
    return op


def _build():
    from contextlib import ExitStack

    import concourse.mybir as mybir
    import concourse.tile as tile
    from concourse import bacc
    from concourse.bass import ts

    f32 = mybir.dt.float32
    f16 = mybir.dt.float16
    i32 = mybir.dt.int32
    i16 = mybir.dt.int16
    Exp = mybir.ActivationFunctionType.Exp
    Mult = mybir.AluOpType.mult
    Add = mybir.AluOpType.add

    polish_op = _register_exp_polish()

    nc = bacc.Bacc()
    xT = nc.dram_tensor("xT", [D, T], f16, kind="ExternalInput")
    wq = nc.dram_tensor("wq", [D, F], f16, kind="ExternalInput")
    wk = nc.dram_tensor("wk", [D, F], f16, kind="ExternalInput")
    # wv/bv come pre-interleaved from the host: column h*65+64 is a zero
    # weight column whose bias is 1.0, producing the [V_h | 1] layout that
    # supplies the softmax-denominator row of the PV matmul for free.
    wv = nc.dram_tensor("wv", [D, VF], f16, kind="ExternalInput")
    bq = nc.dram_tensor("bq", [F, 1], f32, kind="ExternalInput")
    bk = nc.dram_tensor("bk", [F, 1], f32, kind="ExternalInput")
    bv = nc.dram_tensor("bv", [1, VF], f16, kind="ExternalInput")
    wo = nc.dram_tensor("wo", [F, D], f16, kind="ExternalInput")
    out = nc.dram_tensor("out", [T, D], f16, kind="ExternalOutput")

    with ExitStack() as ctx:
        tc = ctx.enter_context(tile.TileContext(nc))
        pers = ctx.enter_context(tc.tile_pool(name="pers", bufs=1))
        ptp = ctx.enter_context(tc.tile_pool(name="ptp", bufs=2))
        p0p = ctx.enter_context(tc.tile_pool(name="p0p", bufs=2))
        osb = ctx.enter_context(tc.tile_pool(name="osb", bufs=2))
        msc = ctx.enter_context(tc.tile_pool(name="msc", bufs=2))
        psq = ctx.enter_context(tc.tile_pool(name="psq", bufs=2, space="PSUM"))
        pss = ctx.enter_context(tc.tile_pool(name="pss", bufs=2, space="PSUM"))
        pso = ctx.enter_context(tc.tile_pool(name="pso", bufs=1, space="PSUM"))

        xt = pers.tile([128, KTN, T], f16, tag="xt")
        wqs = pers.tile([128, KTN, F], f16, tag="wqs")
        wks = pers.tile([128, KTN, F], f16, tag="wks")
        wvs = pers.tile([128, KTN, VF], f16, tag="wvs")
        bqc = pers.tile([128, FT, 1], f32, tag="bqc")
        bkc = pers.tile([128, FT, 1], f32, tag="bkc")
        bvr = pers.tile([1, VF], f16, tag="bvr")
        ones_f = pers.tile([1, 128], f32, tag="ones_f")
        ones16 = pers.tile([1, 128], f16, tag="ones16")
        maskc = pers.tile([128, 1], i32, tag="maskc")
        pcc = pers.tile([128, 1], f32, tag="pcc")
        wos = pers.tile([128, FT, D], f16, tag="wos")
        qt = pers.tile([128, FT, T], f16, tag="qt")
        kt = pers.tile([128, FT, T], f16, tag="kt")
        vs = pers.tile([128, TT, VF], f16, tag="vs")
        at = pers.tile([128, FT, T], f16, tag="at")

        # ISA memset can't target f16; memset f32 then copy-convert
        nc.vector.memset(ones_f[:], 1.0)
        nc.vector.tensor_copy(ones16[:], ones_f[:])
        nc.vector.memset(maskc[:], EXP_MASK)
        nc.vector.memset(pcc[:], EXP_PC)

        # ---- front loads: wv/x/wk first (V tiles + K proj unblock the
        # pipeline), weights on the gpsimd queue, x on sync ----
        nc.gpsimd.dma_start(bvr[:], bv[:])
        for k in range(KTN):
            nc.gpsimd.dma_start(wvs[:, k, :], wv[ts(k, 128), :])
            nc.sync.dma_start(xt[:, k, :], xT[ts(k, 128), :])
            nc.gpsimd.dma_start(wks[:, k, :], wk[ts(k, 128), :])
        for ft in range(FT):
            nc.gpsimd.dma_start(bkc[:, ft, :], bk[ts(ft, 128), :])
        for k in range(KTN):
            nc.gpsimd.dma_start(wqs[:, k, :], wq[ts(k, 128), :])
        for ft in range(FT):
            nc.gpsimd.dma_start(bqc[:, ft, :], bq[ts(ft, 128), :])
        for ft in range(FT):
            nc.gpsimd.dma_start(wos[:, ft, :], wo[ts(ft, 128), :])

        # ---- deferred work units (emitted inside attention j-loops) ----
        def qk_chunk(wsb, bcol, dst, ft, c):
            def go():
                ps = psq.tile([128, NCH], f32, tag="psq", name="ps")
                for k in range(KTN):
                    nc.tensor.matmul(
                        ps[:],
                        wsb[:, k, ts(ft, 128)],
                        xt[:, k, ts(c, NCH)],
                        start=(k == 0), stop=(k == KTN - 1),
                    )
                nc.vector.tensor_scalar_add(
                    dst[:, ft, ts(c, NCH)], ps[:], bcol[:, ft, :]
                )
            return go

        def v_tile(t):
            def go():
                pv = psq.tile([128, VF], f32, tag="psq", name="pv")
                for k in range(KTN):
                    nc.tensor.matmul(
                        pv[:], xt[:, k, ts(t, 128)], wvs[:, k, :],
                        start=(k == 0), stop=False,
                    )
                # bias via ones-row (also writes the denominator 1.0 cols)
                nc.tensor.matmul(
                    pv[:], ones16[:, 0:128], bvr[:], start=False, stop=True
                )
                nc.scalar.copy(vs[:, t, :], pv[:])
            return go

        def outproj_tile(t):
            def go():
                ob = osb.tile([128, D], f16, tag="ob", name="ob")
                for c in range(D // NCH):
                    pp = psq.tile([128, NCH], f32, tag="psq", name="pp")
                    for ft in range(FT):
                        nc.tensor.matmul(
                            pp[:],
                            at[:, ft, ts(t, 128)],
                            wos[:, ft, ts(c, NCH)],
                            start=(ft == 0), stop=(ft == FT - 1),
                        )
                    nc.scalar.copy(ob[:, ts(c, NCH)], pp[:])
                nc.sync.dma_start(out[ts(t, 128), :], ob[:])
            return go

        def make_norm(p, ic, accs):
            """Softmax normalization for group (p, ic): attnT = num/denom.
            Emitted a few iterations into the NEXT group so its DVE work
            never stalls the exp stream at group boundaries."""
            def go():
                dst_sl = ts(ic, NCH)
                for hh in range(2):
                    acc = accs[hh]
                    # denominator row 64 -> partition 0 (custom-DVE ops
                    # drop the partition base offset); copy on ACT
                    dn = msc.tile([1, NCH], f32, tag="dn", bufs=2)
                    nc.scalar.copy(dn[:], acc[DH: DH + 1, :])
                    rc = msc.tile([1, NCH], f32, tag="rc", bufs=2)
                    nc.vector.reciprocal_approx_fast(rc[:], dn[:])
                    rcr = msc.tile([1, NCH], f16, tag="rcr", bufs=2)
                    nc.vector.tensor_copy(rcr[:], rc[:])  # round to f16
                    # broadcast partition 0 -> 64 partitions on idle GPSIMD
                    bsb = msc.tile([DH, NCH], f16, tag="bsb")
                    nc.gpsimd.partition_broadcast(bsb[:], rcr[:], channels=DH)
                    if hh == 0:
                        nc.vector.tensor_mul(
                            at[0:DH, p, dst_sl], acc[0:DH, :], bsb[:]
                        )
                    else:
                        # DVE lanes can't shift partitions; bounce via DMA
                        tmp = msc.tile([DH, NCH], f16, tag="tmp", bufs=2)
                        nc.vector.tensor_mul(tmp[:], acc[0:DH, :], bsb[:])
                        nc.sync.dma_start(at[64:128, p, dst_sl], tmp[:])
            return go

        def make_scores(p, ic):
            def scores(j):
                # disjoint PE row groups (partitions 0-63 / 64-127): the two
                # K=64 matmuls execute concurrently
                sc = pss.tile([128, 2 * NCH], f32, tag="sc", name="sc")
                for hh in range(2):
                    nc.tensor.matmul(
                        sc[:, ts(hh, NCH)],
                        kt[hh * 64: hh * 64 + DH, p, ts(j, 128)],
                        qt[hh * 64: hh * 64 + DH, p, ts(ic, NCH)],
                        start=True, stop=True,
                    )
                return sc
            return scores

        def emit_exp(pe, sc, j):
            if j in DVE_JS:
                pe0 = p0p.tile([128, 2 * NCH], f16, tag="pe0", name="pe0")
                nc.vector.tensor_scalar(
                    pe0[:].bitcast(i16), sc[:], EXP_ALPHA, EXP_BETA,
                    op0=Mult, op1=Add,
                )
                nc.vector._custom_dve(
                    polish_op, out=pe[:], in0=pe0[:], in1=pcc[:],
                    s0=maskc[:].bitcast(f32), s1=1.0, imm2=EXP_PB,
                )
            else:
                nc.scalar.activation(pe[:], sc[:], Exp, scale=0.125)

        seq = [(p, ic) for p in range(FT) for ic in range(T // NCH)]
        scores_of = {g: make_scores(*g) for g in seq}

        # filler schedule: extras[(gi, j)] = list of thunks
        extras = {}
        def add(gi, j, th):
            extras.setdefault((gi, j), []).append(th)

        for j in range(TT - 1):                     # g0: V proj just-in-time
            add(0, j, v_tile(j + 1))
        add(0, 1, qk_chunk(wks, bkc, kt, 0, 1))
        add(0, 5, qk_chunk(wks, bkc, kt, 0, 2))
        add(0, 9, qk_chunk(wks, bkc, kt, 0, 3))
        add(0, 13, qk_chunk(wqs, bqc, qt, 0, 1))
        add(1, 0, qk_chunk(wks, bkc, kt, 1, 0))
        add(1, 2, qk_chunk(wqs, bqc, qt, 0, 2))
        add(1, 4, qk_chunk(wks, bkc, kt, 1, 1))
        add(1, 8, qk_chunk(wks, bkc, kt, 1, 2))
        add(1, 12, qk_chunk(wks, bkc, kt, 1, 3))
        add(2, 2, qk_chunk(wqs, bqc, qt, 0, 3))
        add(2, 6, qk_chunk(wqs, bqc, qt, 1, 0))
        add(2, 10, qk_chunk(wqs, bqc, qt, 1, 1))
        add(3, 4, qk_chunk(wqs, bqc, qt, 1, 2))
        add(3, 8, qk_chunk(wqs, bqc, qt, 1, 3))
        for i in range(4):                          # out-proj, one ic behind
            add(5, 4 + 3 * i, outproj_tile(i))
            add(6, 4 + 3 * i, outproj_tile(4 + i))
            add(7, 4 + 3 * i, outproj_tile(8 + i))

        # ---- prologue: just enough projection for the first group ----
        v_tile(0)()
        qk_chunk(wks, bkc, kt, 0, 0)()
        qk_chunk(wqs, bqc, qt, 0, 0)()

        # ---- flat attention pipeline over all 8 groups ----
        sc_cur = scores_of[seq[0]](0)
        for gi, (p, ic) in enumerate(seq):
            acc0 = pso.tile([VW, NCH], f32, tag="acc0", name="acc0")
            acc1 = pso.tile([VW, NCH], f32, tag="acc1", name="acc1")
            accs = (acc0, acc1)
            for j in range(TT):
                pe = ptp.tile([128, 2 * NCH], f16, tag="pe", name="pe")
                emit_exp(pe, sc_cur, j)
                if j + 1 < TT:
                    sc_cur = scores_of[(p, ic)](j + 1)
                elif gi + 1 < len(seq):
                    sc_cur = scores_of[seq[gi + 1]](0)  # no exp-stream break
                for hh in range(2):
                    nc.tensor.matmul(
                        accs[hh][:, :],
                        vs[:, j, (2 * p + hh) * VW: (2 * p + hh + 1) * VW],
                        pe[:, ts(hh, NCH)],
                        start=(j == 0), stop=(j == TT - 1),
                    )
                for th in extras.get((gi, j), ()):
                    th()
            # normalization runs inside the next group (j==2) so it
            # overlaps that group's compute; last group: emit now
            if gi + 1 < len(seq):
                add(gi + 1, 2, make_norm(p, ic, accs))
            else:
                make_norm(p, ic, accs)()
        for t in range(12, 16):
            outproj_tile(t)()

    nc.finalize()  # Bacc.compile(): wait legalization, reg alloc, act tables
    return nc


def _get_program():
    global _prog
    if _prog is None:
        _prog = _build()
    return _prog


def kernel(x, W_qkv, b_qkv, W_out, b_out):
    global LAST_RESULT
    from concourse.bass_utils import run_bass_kernel_spmd

    x = np.asarray(x, np.float32)
    W_qkv = np.asarray(W_qkv, np.float32)
    b_qkv = np.asarray(b_qkv, np.float32)
    W_out = np.asarray(W_out, np.float32)
    b_out = np.asarray(b_out, np.float32)

    nc = _get_program()

    in_maps = []
    for c in range(NCORES):
        b, g = divmod(c, GROUPS)
        sl = slice(g * F, (g + 1) * F)
        # interleave Wv/bv with [zero-weight, bias=1] columns at h*65+64
        wv_g = W_qkv[:, 2 * D:3 * D][:, sl]
        bv_g = b_qkv[2 * D:3 * D][sl]
        wv_i = np.zeros((D, VF), np.float16)
        bv_i = np.zeros((1, VF), np.float16)
        for h in range(HPC):
            wv_i[:, h * VW: h * VW + DH] = wv_g[:, h * DH:(h + 1) * DH]
            bv_i[0, h * VW: h * VW + DH] = bv_g[h * DH:(h + 1) * DH]
            bv_i[0, h * VW + DH] = 1.0
        in_maps.append({
            "xT": np.ascontiguousarray(x[b].T.astype(np.float16)),
            "wq": np.ascontiguousarray(W_qkv[:, 0 * D:1 * D][:, sl]).astype(np.float16),
            "wk": np.ascontiguousarray(W_qkv[:, 1 * D:2 * D][:, sl]).astype(np.float16),
            "wv": wv_i,
            "bq": np.ascontiguousarray(b_qkv[0 * D:1 * D][sl][:, None]),
            "bk": np.ascontiguousarray(b_qkv[1 * D:2 * D][sl][:, None]),
            "bv": bv_i,
            "wo": np.ascontiguousarray(W_out[sl, :]).astype(np.float16),
        })

    kw = {}
    if os.environ.get("KERNEL_TRACE") == "1":
        kw["trace"] = True
    res = run_bass_kernel_spmd(nc, in_maps, core_ids=list(range(NCORES)), **kw)
    LAST_RESULT = res

    out = np.empty((B, T, D), np.float32)
    for b in range(B):
        acc = res.results[GROUPS * b]["out"].astype(np.float32)
        for g in range(1, GROUPS):
            acc = acc + res.results[GROUPS * b + g]["out"].astype(np.float32)
        out[b] = acc + b_out
    return out


# revision 9
# speedup vs baseline: 1.0391x; 1.0391x over previous
"""Multi-head attention (B=2, T=2048, D=1024, H=16, Dh=64) on 8 TRN2 NeuronCores.

Sharding: core c = 4*b + g  ->  batch b in {0,1}, head-group g in {0..3}
(4 heads per core: data parallel on batch, tensor parallel on heads).
Each core computes, for its batch element and its 4 heads:

  Q.T/K.T = Wq/k_shard.T @ x.T + b      [256, 2048]  (head-dim on partitions)
  V'      = x @ Wv_interleaved + b      [2048, 260]  ([V_h | 1] per head)
  per head pair (2p, 2p+1), per 512-wide i-chunk:
    S.T   = K_h Q_h.T                   (two K=64 matmuls on disjoint PE
                                         row groups -> run concurrently)
    P.T   = exp(S.T / 8)                (no max-subtraction: |S|/8 <~ 6)
    acc   = [V_h | 1].T @ P.T           [65, 512]  row 64 = softmax denom
    attnT = acc[:64] * (1/acc[64])
  partial = attnT.T @ Wout_shard        [2048, 1024]  (f16 out, host-summed)

The partial sum over the 4 head groups plus b_out is done on the host
("all-reduce after out_proj"), as is the batch unshard.

The softmax exp stream is split across two engines so neither serializes
the kernel: the scalar engine (ACT) computes true exp for most j-tiles;
for j in DVE_JS the vector engine computes a bit-trick exp2 -- one
tensor_scalar (x*a+b -> int16 bit pattern read as f16) plus one custom
DVE op that polishes the mantissa-linear error with a quadratic in
m = 1+frac (recovered exactly via bitwise and/or), max rel err ~5e-3.
PSUM evictions (V tiles, out-proj tiles, denominator rows) run on the
scalar engine; softmax normalization runs on the vector engine with the
reciprocal row broadcast across partitions by a 0-stride DMA.
"""

import os
import numpy as np

B, T, D = 2, 2048, 1024
H, DH = 16, 64
NCORES, GROUPS = 8, 4
HPC = H // GROUPS        # 4 heads per core
F = HPC * DH             # 256 features per core
FT = F // 128            # 2 feature tiles / head pairs
KTN = D // 128           # 8 contraction tiles
TT = T // 128            # 16 token tiles
NCH = 512                # matmul free-dim chunk
VW = DH + 1              # 65: V plus ones column
VF = HPC * VW            # 260: interleaved [V_h | 1] x 4 heads

# exp split: j-tiles per group computed on the DVE instead of ACT.
# First-half groups carry qk-bias adds on the DVE (5 tiles); second-half
# groups carry out-proj evictions on ACT (6 tiles).
DVE_JS_H1 = frozenset({2, 5, 8, 11, 14})
DVE_JS_H2 = frozenset({2, 4, 7, 9, 12, 14})

# f16 Schraudolph constants: i16 bits = round(s*ALPHA + BETA), then
# polish out = pe0 * ((m + PB) * m + PC), m = or(and(bits, MASK), 1.0).
# BETA includes the polish normalization (c0_adj = 2.0496...).
EXP_ALPHA = 184.6649652337873
EXP_BETA = 13261.182453842255
EXP_PB = -2.960537740957013
EXP_PC = 6.080904660347446
EXP_MASK = 0x007FE000

_prog = None
LAST_RESULT = None


def _register_exp_polish():
    """Register the EXP_POLISH_ANT custom DVE op (quadratic mantissa polish)."""
    import concourse.dve_ops as dom

    name = "EXP_POLISH_ANT"
    for op in dom.OPS:
        if op.name == name:
            return op
    from concourse.dve_spec import (
        Spec, Src0, C0, C1, C2, C3, Bin, AluOp, _spill_c3_to_src1, _has_src1, lower,
    )
    from concourse.dve_uop import DveOpSpec

    m = Bin(AluOp.BITWISE_OR, Bin(AluOp.BITWISE_AND, Src0, C0), C1)
    body = _spill_c3_to_src1(Src0 * ((m + C2) * m + C3))

    def ref(in0, in1, s0, s1, imm2):
        x = np.asarray(in0, np.float32)
        mask = np.asarray(s0, np.float32).view(np.uint32)
        orv = np.asarray(s1, np.float32).view(np.uint32)
        mm = ((x.view(np.uint32) & mask) | orv).view(np.float32)
        c = np.asarray(in1, np.float32)
        return (x * ((mm + np.float32(imm2)) * mm + c)).astype(np.float32)

    spec = Spec(body=body, reference=ref)
    row = max(dom._SUB_OPCODE_FOR_NAME.values()) + 1
    assert row < 0x20, "no free custom-DVE opcode row"
    dom._SUB_OPCODE_FOR_NAME[name] = row
    shas = {}
    for ver in ("v3", "v4"):
        try:
            shas[ver] = DveOpSpec(
                name=name, opcode=row, uops=lower(spec, ver=ver),
                rd1_en=_has_src1(spec),
            ).sha(ver)
        except Exception:
            if ver == "v3":
                raise  # TRN2 is v3; v4 is best-effort
    op = dom.DveOp(name, spec, subdim=False, uops_sha=shas)
    dom.OPS.append(op)
    dom.CUSTOM_DVE_SPECS[name] = spec
    return op


def _build():
    from contextlib import ExitStack

    import concourse.mybir as mybir
    import concourse.tile as tile
    from concourse import bacc
    from concourse.bass import ts

    f32 = mybir.dt.float32
    f16 = mybir.dt.float16
    i32 = mybir.dt.int32
    i16 = mybir.dt.int16
    Exp = mybir.ActivationFunctionType.Exp
    Mult = mybir.AluOpType.mult
    Add = mybir.AluOpType.add

    polish_op = _register_exp_polish()

    nc = bacc.Bacc()
    xT = nc.dram_tensor("xT", [D, T], f16, kind="ExternalInput")
    wq = nc.dram_tensor("wq", [D, F], f16, kind="ExternalInput")
    wk = nc.dram_tensor("wk", [D, F], f16, kind="ExternalInput")
    # wv/bv come pre-interleaved from the host: column h*65+64 is a zero
    # weight column whose bias is 1.0, producing the [V_h | 1] layout that
    # supplies the softmax-denominator row of the PV matmul for free.
    wv = nc.dram_tensor("wv", [D, VF], f16, kind="ExternalInput")
    bq = nc.dram_tensor("bq", [F, 1], f32, kind="ExternalInput")
    bk = nc.dram_tensor("bk", [F, 1], f32, kind="ExternalInput")
    bv = nc.dram_tensor("bv", [1, VF], f16, kind="ExternalInput")
    wo = nc.dram_tensor("wo", [F, D], f16, kind="ExternalInput")
    out = nc.dram_tensor("out", [T, D], f16, kind="ExternalOutput")

    with ExitStack() as ctx:
        tc = ctx.enter_context(tile.TileContext(nc))
        pers = ctx.enter_context(tc.tile_pool(name="pers", bufs=1))
        ptp = ctx.enter_context(tc.tile_pool(name="ptp", bufs=2))
        p0p = ctx.enter_context(tc.tile_pool(name="p0p", bufs=2))
        osb = ctx.enter_context(tc.tile_pool(name="osb", bufs=2))
        msc = ctx.enter_context(tc.tile_pool(name="msc", bufs=2))
        psq = ctx.enter_context(tc.tile_pool(name="psq", bufs=2, space="PSUM"))
        pss = ctx.enter_context(tc.tile_pool(name="pss", bufs=2, space="PSUM"))
        pso = ctx.enter_context(tc.tile_pool(name="pso", bufs=1, space="PSUM"))

        xt = pers.tile([128, KTN, T], f16, tag="xt")
        wqs = pers.tile([128, KTN, F], f16, tag="wqs")
        wks = pers.tile([128, KTN, F], f16, tag="wks")
        wvs = pers.tile([128, KTN, VF], f16, tag="wvs")
        bqc = pers.tile([128, FT, 1], f32, tag="bqc")
        bkc = pers.tile([128, FT, 1], f32, tag="bkc")
        bvr = pers.tile([1, VF], f16, tag="bvr")
        ones_f = pers.tile([1, 128], f32, tag="ones_f")
        ones16 = pers.tile([1, 128], f16, tag="ones16")
        maskc = pers.tile([128, 1], i32, tag="maskc")
        pcc = pers.tile([128, 1], f32, tag="pcc")
        wos = pers.tile([128, FT, D], f16, tag="wos")
        qt = pers.tile([128, FT, T], f16, tag="qt")
        kt = pers.tile([128, FT, T], f16, tag="kt")
        vs = pers.tile([128, TT, VF], f16, tag="vs")
        at = pers.tile([128, FT, T], f16, tag="at")

        # ISA memset can't target f16; memset f32 then copy-convert
        nc.vector.memset(ones_f[:], 1.0)
        nc.vector.tensor_copy(ones16[:], ones_f[:])
        nc.vector.memset(maskc[:], EXP_MASK)
        nc.vector.memset(pcc[:], EXP_PC)

        # ---- front loads: weights on the gpsimd queue, x on sync, both in
        # first-use order. xt comes in two half-T waves so the first scores
        # group (cols 0:1024) unblocks after ~2MB instead of the full 4MB.
        nc.gpsimd.dma_start(bvr[:], bv[:])
        for k in range(KTN):
            nc.gpsimd.dma_start(wvs[:, k, :], wv[ts(k, 128), :])
            nc.sync.dma_start(xt[:, k, 0:T // 2], xT[ts(k, 128), 0:T // 2])
            nc.gpsimd.dma_start(wks[:, k, :], wk[ts(k, 128), :])
        for ft in range(FT):
            nc.gpsimd.dma_start(bkc[:, ft, :], bk[ts(ft, 128), :])
        for k in range(KTN):
            nc.gpsimd.dma_start(wqs[:, k, :], wq[ts(k, 128), :])
            nc.sync.dma_start(xt[:, k, T // 2:T], xT[ts(k, 128), T // 2:T])
        for ft in range(FT):
            nc.gpsimd.dma_start(bqc[:, ft, :], bq[ts(ft, 128), :])
        for ft in range(FT):
            nc.gpsimd.dma_start(wos[:, ft, :], wo[ts(ft, 128), :])

        # ---- deferred work units (emitted inside attention j-loops) ----
        def qk_chunk(wsb, bcol, dst, ft, c):
            def go():
                ps = psq.tile([128, NCH], f32, tag="psq", name="ps")
                for k in range(KTN):
                    nc.tensor.matmul(
                        ps[:],
                        wsb[:, k, ts(ft, 128)],
                        xt[:, k, ts(c, NCH)],
                        start=(k == 0), stop=(k == KTN - 1),
                    )
                nc.vector.tensor_scalar_add(
                    dst[:, ft, ts(c, NCH)], ps[:], bcol[:, ft, :]
                )
            return go

        def v_tile(t):
            def go():
                pv = psq.tile([128, VF], f32, tag="psq", name="pv")
                for k in range(KTN):
                    nc.tensor.matmul(
                        pv[:], xt[:, k, ts(t, 128)], wvs[:, k, :],
                        start=(k == 0), stop=False,
                    )
                # bias via ones-row (also writes the denominator 1.0 cols)
                nc.tensor.matmul(
                    pv[:], ones16[:, 0:128], bvr[:], start=False, stop=True
                )
                nc.scalar.copy(vs[:, t, :], pv[:])
            return go

        def outproj_tile(t):
            def go():
                ob = osb.tile([128, D], f16, tag="ob", name="ob")
                for c in range(D // NCH):
                    pp = psq.tile([128, NCH], f32, tag="psq", name="pp")
                    for ft in range(FT):
                        nc.tensor.matmul(
                            pp[:],
                            at[:, ft, ts(t, 128)],
                            wos[:, ft, ts(c, NCH)],
                            start=(ft == 0), stop=(ft == FT - 1),
                        )
                    nc.scalar.copy(ob[:, ts(c, NCH)], pp[:])
                nc.sync.dma_start(out[ts(t, 128), :], ob[:])
            return go

        def make_norm(p, ic, accs):
            """Softmax normalization for group (p, ic): attnT = num/denom.
            Emitted a few iterations into the NEXT group so its DVE work
            never stalls the exp stream at group boundaries."""
            def go():
                dst_sl = ts(ic, NCH)
                for hh in range(2):
                    acc = accs[hh]
                    # denominator row 64 -> partition 0 (custom-DVE ops
                    # drop the partition base offset); copy on ACT
                    dn = msc.tile([1, NCH], f32, tag="dn", bufs=2)
                    nc.scalar.copy(dn[:], acc[DH: DH + 1, :])
                    rc = msc.tile([1, NCH], f32, tag="rc", bufs=2)
                    nc.vector.reciprocal_approx_fast(rc[:], dn[:])
                    rcr = msc.tile([1, NCH], f16, tag="rcr", bufs=2)
                    nc.vector.tensor_copy(rcr[:], rc[:])  # round to f16
                    # broadcast partition 0 -> 64 partitions on idle GPSIMD
                    bsb = msc.tile([DH, NCH], f16, tag="bsb")
                    nc.gpsimd.partition_broadcast(bsb[:], rcr[:], channels=DH)
                    if hh == 0:
                        nc.vector.tensor_mul(
                            at[0:DH, p, dst_sl], acc[0:DH, :], bsb[:]
                        )
                    else:
                        # DVE lanes can't shift partitions; bounce via DMA
                        tmp = msc.tile([DH, NCH], f16, tag="tmp", bufs=2)
                        nc.vector.tensor_mul(tmp[:], acc[0:DH, :], bsb[:])
                        nc.sync.dma_start(at[64:128, p, dst_sl], tmp[:])
            return go

        def make_scores(p, ic):
            def scores(j):
                # disjoint PE row groups (partitions 0-63 / 64-127): the two
                # K=64 matmuls execute concurrently
                sc = pss.tile([128, 2 * NCH], f32, tag="sc", name="sc")
                for hh in range(2):
                    nc.tensor.matmul(
                        sc[:, ts(hh, NCH)],
                        kt[hh * 64: hh * 64 + DH, p, ts(j, 128)],
                        qt[hh * 64: hh * 64 + DH, p, ts(ic, NCH)],
                        start=True, stop=True,
                    )
                return sc
            return scores

        def emit_exp(pe, sc, j, dve_js):
            if j in dve_js:
                pe0 = p0p.tile([128, 2 * NCH], f16, tag="pe0", name="pe0")
                nc.vector.tensor_scalar(
                    pe0[:].bitcast(i16), sc[:], EXP_ALPHA, EXP_BETA,
                    op0=Mult, op1=Add,
                )
                nc.vector._custom_dve(
                    polish_op, out=pe[:], in0=pe0[:], in1=pcc[:],
                    s0=maskc[:].bitcast(f32), s1=1.0, imm2=EXP_PB,
                )
            else:
                nc.scalar.activation(pe[:], sc[:], Exp, scale=0.125)

        seq = [(p, ic) for p in range(FT) for ic in range(T // NCH)]
        scores_of = {g: make_scores(*g) for g in seq}

        # filler schedule: extras[(gi, j)] = list of thunks
        extras = {}
        def add(gi, j, th):
            extras.setdefault((gi, j), []).append(th)

        for j in range(TT - 1):                     # g0: V proj just-in-time
            add(0, j, v_tile(j + 1))
        add(0, 1, qk_chunk(wks, bkc, kt, 0, 1))
        add(0, 5, qk_chunk(wks, bkc, kt, 0, 2))
        add(0, 9, qk_chunk(wks, bkc, kt, 0, 3))
        add(0, 13, qk_chunk(wqs, bqc, qt, 0, 1))
        add(1, 0, qk_chunk(wks, bkc, kt, 1, 0))
        add(1, 2, qk_chunk(wqs, bqc, qt, 0, 2))
        add(1, 4, qk_chunk(wks, bkc, kt, 1, 1))
        add(1, 8, qk_chunk(wks, bkc, kt, 1, 2))
        add(1, 12, qk_chunk(wks, bkc, kt, 1, 3))
        add(2, 2, qk_chunk(wqs, bqc, qt, 0, 3))
        add(2, 6, qk_chunk(wqs, bqc, qt, 1, 0))
        add(2, 10, qk_chunk(wqs, bqc, qt, 1, 1))
        add(3, 4, qk_chunk(wqs, bqc, qt, 1, 2))
        add(3, 8, qk_chunk(wqs, bqc, qt, 1, 3))
        for i in range(4):                          # out-proj, one ic behind
            add(5, 5 + 3 * i, outproj_tile(i))
            add(6, 5 + 3 * i, outproj_tile(4 + i))
            add(7, 5 + 3 * i, outproj_tile(8 + i))

        # ---- prologue: just enough projection for the first group ----
        v_tile(0)()
        qk_chunk(wks, bkc, kt, 0, 0)()
        qk_chunk(wqs, bqc, qt, 0, 0)()

        # ---- flat attention pipeline over all 8 groups ----
        sc_cur = scores_of[seq[0]](0)
        for gi, (p, ic) in enumerate(seq):
            dve_js = DVE_JS_H1 if gi < 4 else DVE_JS_H2
            acc0 = pso.tile([VW, NCH], f32, tag="acc0", name="acc0")
            acc1 = pso.tile([VW, NCH], f32, tag="acc1", name="acc1")
            accs = (acc0, acc1)
            for j in range(TT):
                pe = ptp.tile([128, 2 * NCH], f16, tag="pe", name="pe")
                emit_exp(pe, sc_cur, j, dve_js)
                if j + 1 < TT:
                    sc_cur = scores_of[(p, ic)](j + 1)
                elif gi + 1 < len(seq):
                    sc_cur = scores_of[seq[gi + 1]](0)  # no exp-stream break
                for hh in range(2):
                    nc.tensor.matmul(
                        accs[hh][:, :],
                        vs[:, j, (2 * p + hh) * VW: (2 * p + hh + 1) * VW],
                        pe[:, ts(hh, NCH)],
                        start=(j == 0), stop=(j == TT - 1),
                    )
                for th in extras.get((gi, j), ()):
                    th()
            # normalization runs inside the next group (j==0) so it
            # overlaps that group's compute; last group: emit now
            if gi + 1 < len(seq):
                add(gi + 1, 0, make_norm(p, ic, accs))
            else:
                make_norm(p, ic, accs)()
        for t in range(12, 16):
            outproj_tile(t)()

    nc.finalize()  # Bacc.compile(): wait legalization, reg alloc, act tables
    return nc


def _get_program():
    global _prog
    if _prog is None:
        _prog = _build()
    return _prog


def kernel(x, W_qkv, b_qkv, W_out, b_out):
    global LAST_RESULT
    from concourse.bass_utils import run_bass_kernel_spmd

    x = np.asarray(x, np.float32)
    W_qkv = np.asarray(W_qkv, np.float32)
    b_qkv = np.asarray(b_qkv, np.float32)
    W_out = np.asarray(W_out, np.float32)
    b_out = np.asarray(b_out, np.float32)

    nc = _get_program()

    in_maps = []
    for c in range(NCORES):
        b, g = divmod(c, GROUPS)
        sl = slice(g * F, (g + 1) * F)
        # interleave Wv/bv with [zero-weight, bias=1] columns at h*65+64
        wv_g = W_qkv[:, 2 * D:3 * D][:, sl]
        bv_g = b_qkv[2 * D:3 * D][sl]
        wv_i = np.zeros((D, VF), np.float16)
        bv_i = np.zeros((1, VF), np.float16)
        for h in range(HPC):
            wv_i[:, h * VW: h * VW + DH] = wv_g[:, h * DH:(h + 1) * DH]
            bv_i[0, h * VW: h * VW + DH] = bv_g[h * DH:(h + 1) * DH]
            bv_i[0, h * VW + DH] = 1.0
        in_maps.append({
            "xT": np.ascontiguousarray(x[b].T.astype(np.float16)),
            "wq": np.ascontiguousarray(W_qkv[:, 0 * D:1 * D][:, sl]).astype(np.float16),
            "wk": np.ascontiguousarray(W_qkv[:, 1 * D:2 * D][:, sl]).astype(np.float16),
            "wv": wv_i,
            "bq": np.ascontiguousarray(b_qkv[0 * D:1 * D][sl][:, None]),
            "bk": np.ascontiguousarray(b_qkv[1 * D:2 * D][sl][:, None]),
            "bv": bv_i,
            "wo": np.ascontiguousarray(W_out[sl, :]).astype(np.float16),
        })

    kw = {}
    if os.environ.get("KERNEL_TRACE") == "1":
        kw["trace"] = True
    res = run_bass_kernel_spmd(nc, in_maps, core_ids=list(range(NCORES)), **kw)
    LAST_RESULT = res

    out = np.empty((B, T, D), np.float32)
    for b in range(B):
        acc = res.results[GROUPS * b]["out"].astype(np.float32)
        for g in range(1, GROUPS):
            acc = acc + res.results[GROUPS * b + g]["out"].astype(np.float32)
        out[b] = acc + b_out
    return out


# revision 22
# speedup vs baseline: 1.1186x; 1.0765x over previous
"""Multi-head attention (B=2, T=2048, D=1024, H=16, Dh=64) on 8 TRN2 NeuronCores.

Sharding: core c = 4*b + g  ->  batch b in {0,1}, head-group g in {0..3}
(4 heads per core: data parallel on batch, tensor parallel on heads).
Each core computes, for its batch element and its 4 heads:

  Q.T/K.T = Wq/k_shard.T @ x.T + b      [256, 2048]  (head-dim on partitions)
  V'      = x @ Wv_interleaved + b      [2048, 260]  ([V_h | 1] per head)
  per head pair (2p, 2p+1), per 512-wide i-chunk:
    S.T   = K_h Q_h.T                   (two K=64 matmuls on disjoint PE
                                         row groups -> run concurrently)
    P.T   = exp(S.T / 8)                (no max-subtraction: |S|/8 <~ 6)
    acc   = [V_h | 1].T @ P.T           [65, 512]  row 64 = softmax denom
    attnT = acc[:64] * (1/acc[64])
  partial = attnT.T @ Wout_shard        [2048, 1024]  (f16 out, host-summed)

The partial sum over the 4 head groups plus b_out is done on the host
("all-reduce after out_proj"), as is the batch unshard.

The softmax exp stream is split across two engines so neither serializes
the kernel: the scalar engine (ACT) computes true exp for most j-tiles;
for j in DVE_JS the vector engine computes a bit-trick exp2 -- one
tensor_scalar (x*a+b -> int16 bit pattern read as f16) plus one custom
DVE op that polishes the mantissa-linear error with a quadratic in
m = 1+frac (recovered exactly via bitwise and/or), max rel err ~5e-3.
PSUM evictions (V tiles, out-proj tiles, denominator rows) run on the
scalar engine; softmax normalization runs on the vector engine with the
reciprocal row broadcast across partitions by a 0-stride DMA.
"""

import os
import numpy as np

B, T, D = 2, 2048, 1024
H, DH = 16, 64
NCORES, GROUPS = 8, 4
HPC = H // GROUPS        # 4 heads per core
F = HPC * DH             # 256 features per core
FT = F // 128            # 2 feature tiles / head pairs
KTN = D // 128           # 8 contraction tiles
TT = T // 128            # 16 token tiles
NCH = 512                # matmul free-dim chunk
VW = DH + 1              # 65: V plus ones column
VF = HPC * VW            # 260: interleaved [V_h | 1] x 4 heads

# exp split: j-tiles per group computed on the DVE instead of ACT.
# Boundary js (14, 15, 0..2) stay on ACT so the scores-PSUM recycle and
# the group-start PV never wait on the slower 2-instruction DVE path.
DVE_JS_H1 = frozenset({3, 5, 7, 9, 11})
DVE_JS_H2 = frozenset({3, 4, 6, 8, 10, 12})
# PV matmuls trail exp emission by PV_TGT[j] tiles. The bulge at j=3..5
# suspends PV pops right after a group boundary: the previous group's
# normalization chain (4 engines deep) runs while the PE chews scores and
# extras instead of stalling on the freshly-recycled acc banks.
PV_TGT = {0: 3, 1: 3, 2: 3, 3: 6, 4: 6, 5: 6, 6: 5, 7: 4}
PV_LAG_MAX = 6

# f16 Schraudolph constants: i16 bits = round(s*ALPHA + BETA), then
# polish out = pe0 * ((m + PB) * m + PC), m = or(and(bits, MASK), 1.0).
# BETA includes the polish normalization (c0_adj = 2.0496...).
EXP_ALPHA = 184.6649652337873
EXP_BETA = 13261.182453842255
EXP_PB = -2.960537740957013
EXP_PC = 6.080904660347446
EXP_MASK = 0x007FE000

_prog = None
LAST_RESULT = None


def _register_exp_polish():
    """Register the EXP_POLISH_ANT custom DVE op (quadratic mantissa polish)."""
    import concourse.dve_ops as dom

    name = "EXP_POLISH_ANT"
    for op in dom.OPS:
        if op.name == name:
            return op
    from concourse.dve_spec import (
        Spec, Src0, C0, C1, C2, C3, Bin, AluOp, _spill_c3_to_src1, _has_src1, lower,
    )
    from concourse.dve_uop import DveOpSpec

    m = Bin(AluOp.BITWISE_OR, Bin(AluOp.BITWISE_AND, Src0, C0), C1)
    body = _spill_c3_to_src1(Src0 * ((m + C2) * m + C3))

    def ref(in0, in1, s0, s1, imm2):
        x = np.asarray(in0, np.float32)
        mask = np.asarray(s0, np.float32).view(np.uint32)
        orv = np.asarray(s1, np.float32).view(np.uint32)
        mm = ((x.view(np.uint32) & mask) | orv).view(np.float32)
        c = np.asarray(in1, np.float32)
        return (x * ((mm + np.float32(imm2)) * mm + c)).astype(np.float32)

    spec = Spec(body=body, reference=ref)
    row = max(dom._SUB_OPCODE_FOR_NAME.values()) + 1
    assert row < 0x20, "no free custom-DVE opcode row"
    dom._SUB_OPCODE_FOR_NAME[name] = row
    shas = {}
    for ver in ("v3", "v4"):
        try:
            shas[ver] = DveOpSpec(
                name=name, opcode=row, uops=lower(spec, ver=ver),
                rd1_en=_has_src1(spec),
            ).sha(ver)
        except Exception:
            if ver == "v3":
                raise  # TRN2 is v3; v4 is best-effort
    op = dom.DveOp(name, spec, subdim=False, uops_sha=shas)
    dom.OPS.append(op)
    dom.CUSTOM_DVE_SPECS[name] = spec
    return op


def _build():
    from contextlib import ExitStack

    import concourse.mybir as mybir
    import concourse.tile as tile
    from concourse import bacc
    from concourse.bass import ts

    f32 = mybir.dt.float32
    f16 = mybir.dt.float16
    i32 = mybir.dt.int32
    i16 = mybir.dt.int16
    Exp = mybir.ActivationFunctionType.Exp
    Mult = mybir.AluOpType.mult
    Add = mybir.AluOpType.add

    polish_op = _register_exp_polish()

    nc = bacc.Bacc()
    xT = nc.dram_tensor("xT", [D, T], f16, kind="ExternalInput")
    wq = nc.dram_tensor("wq", [D, F], f16, kind="ExternalInput")
    wk = nc.dram_tensor("wk", [D, F], f16, kind="ExternalInput")
    # wv/bv come pre-interleaved from the host: column h*65+64 is a zero
    # weight column whose bias is 1.0, producing the [V_h | 1] layout that
    # supplies the softmax-denominator row of the PV matmul for free.
    wv = nc.dram_tensor("wv", [D, VF], f16, kind="ExternalInput")
    bq = nc.dram_tensor("bq", [F, 1], f32, kind="ExternalInput")
    bk = nc.dram_tensor("bk", [F, 1], f32, kind="ExternalInput")
    bv = nc.dram_tensor("bv", [1, VF], f16, kind="ExternalInput")
    wo = nc.dram_tensor("wo", [F, D], f16, kind="ExternalInput")
    out = nc.dram_tensor("out", [T, D], f16, kind="ExternalOutput")

    from concourse.dve_ops import RECIP_APPROX_FAST_CONSTS, RECIPROCAL_APPROX_FAST

    with ExitStack() as ctx:
        tc = ctx.enter_context(tile.TileContext(nc))
        pers = ctx.enter_context(tc.tile_pool(name="pers", bufs=1))
        ptp = ctx.enter_context(tc.tile_pool(name="ptp", bufs=PV_LAG_MAX + 2))
        p0p = ctx.enter_context(tc.tile_pool(name="p0p", bufs=2))
        osb = ctx.enter_context(tc.tile_pool(name="osb", bufs=2))
        msc = ctx.enter_context(tc.tile_pool(name="msc", bufs=2))
        psq = ctx.enter_context(tc.tile_pool(name="psq", bufs=2, space="PSUM"))
        pss = ctx.enter_context(tc.tile_pool(name="pss", bufs=2, space="PSUM"))
        pso = ctx.enter_context(tc.tile_pool(name="pso", bufs=1, space="PSUM"))

        xt = pers.tile([128, KTN, T], f16, tag="xt")
        wqs = pers.tile([128, KTN, F], f16, tag="wqs")
        wks = pers.tile([128, KTN, F], f16, tag="wks")
        wvs = pers.tile([128, KTN, VF], f16, tag="wvs")
        bqc = pers.tile([128, FT, 1], f32, tag="bqc")
        bkc = pers.tile([128, FT, 1], f32, tag="bkc")
        bvr = pers.tile([1, VF], f16, tag="bvr")
        ones_f = pers.tile([1, 128], f32, tag="ones_f")
        ones16 = pers.tile([1, 128], f16, tag="ones16")
        maskc = pers.tile([128, 1], i32, tag="maskc")
        pcc = pers.tile([128, 1], f32, tag="pcc")
        wos = pers.tile([128, FT, D], f16, tag="wos")
        qt = pers.tile([128, FT, T], f16, tag="qt")
        kt = pers.tile([128, FT, T], f16, tag="kt")
        vs = pers.tile([128, TT, VF], f16, tag="vs")
        at = pers.tile([128, FT, T], f16, tag="at")

        # ISA memset can't target f16; memset f32 then copy-convert
        nc.vector.memset(ones_f[:], 1.0)
        nc.vector.tensor_copy(ones16[:], ones_f[:])
        nc.vector.memset(maskc[:], EXP_MASK)
        nc.vector.memset(pcc[:], EXP_PC)

        # ---- front loads: weights on the gpsimd queue, x on sync, both in
        # first-use order. xt comes in three 512-col waves so the first
        # scores group unblocks after ~2.5MB instead of the full 4MB.
        nc.gpsimd.dma_start(bvr[:], bv[:])
        for k in range(KTN):
            nc.gpsimd.dma_start(wvs[:, k, :], wv[ts(k, 128), :])
            nc.sync.dma_start(xt[:, k, 0:NCH], xT[ts(k, 128), 0:NCH])
        for k in range(KTN):
            nc.gpsimd.dma_start(wks[:, k, :], wk[ts(k, 128), :])
            nc.gpsimd.dma_start(wqs[:, k, :], wq[ts(k, 128), :])
            nc.sync.dma_start(xt[:, k, NCH:2 * NCH], xT[ts(k, 128), NCH:2 * NCH])
        for ft in range(FT):
            nc.gpsimd.dma_start(bkc[:, ft, :], bk[ts(ft, 128), :])
            nc.gpsimd.dma_start(bqc[:, ft, :], bq[ts(ft, 128), :])
        for k in range(KTN):
            nc.sync.dma_start(xt[:, k, 2 * NCH:T], xT[ts(k, 128), 2 * NCH:T])
        for ft in range(FT):
            nc.gpsimd.dma_start(wos[:, ft, :], wo[ts(ft, 128), :])

        # ---- PE warmup: ~4us of junk matmuls while the front DMAs land,
        # so the HAM clock gate reaches 8/8 before the real stream starts
        for _ in range(12):
            junk = psq.tile([128, VF], f32, tag="psq", name="junk")
            nc.tensor.matmul(junk[:], ones16[:, 0:128], bvr[:],
                             start=True, stop=True)

        # ---- deferred work units (emitted inside attention j-loops) ----
        def qk_chunk(wsb, bcol, dst, ft, c):
            def go():
                ps = psq.tile([128, NCH], f32, tag="psq", name="ps")
                for k in range(KTN):
                    nc.tensor.matmul(
                        ps[:],
                        wsb[:, k, ts(ft, 128)],
                        xt[:, k, ts(c, NCH)],
                        start=(k == 0), stop=(k == KTN - 1),
                    )
                nc.vector.tensor_scalar_add(
                    dst[:, ft, ts(c, NCH)], ps[:], bcol[:, ft, :]
                )
            return go

        def v_tile(t):
            def go():
                pv = psq.tile([128, VF], f32, tag="psq", name="pv")
                for k in range(KTN):
                    nc.tensor.matmul(
                        pv[:], xt[:, k, ts(t, 128)], wvs[:, k, :],
                        start=(k == 0), stop=False,
                    )
                # bias via ones-row (also writes the denominator 1.0 cols)
                nc.tensor.matmul(
                    pv[:], ones16[:, 0:128], bvr[:], start=False, stop=True
                )
                nc.scalar.copy(vs[:, t, :], pv[:])
            return go

        def outproj_tile(t):
            def go():
                ob = osb.tile([128, D], f16, tag="ob", name="ob")
                # ft-outer so the two matmuls sharing lhsT are back-to-back
                pps = [psq.tile([128, NCH], f32, tag="psq", name="pp")
                       for _ in range(D // NCH)]
                for ft in range(FT):
                    for c in range(D // NCH):
                        nc.tensor.matmul(
                            pps[c][:],
                            at[:, ft, ts(t, 128)],
                            wos[:, ft, ts(c, NCH)],
                            start=(ft == 0), stop=(ft == FT - 1),
                        )
                for c in range(D // NCH):
                    nc.scalar.copy(ob[:, ts(c, NCH)], pps[c][:])
                nc.sync.dma_start(out[ts(t, 128), :], ob[:])
            return go

        def make_norm(p, ic, accs):
            """Softmax normalization for group (p, ic): attnT = num/denom.
            Two stages emitted early in the NEXT group; the PV emission lag
            gives the cross-engine chain time to clear before the next
            group's first PV needs the acc banks back."""
            dst_sl = ts(ic, NCH)
            c = RECIP_APPROX_FAST_CONSTS
            bsbs = []

            def stage_a():
                for hh in range(2):
                    # denominator row 64 -> partition 0 (custom-DVE ops
                    # drop the partition base offset); copy on ACT
                    dn = msc.tile([1, NCH], f32, tag="dn", bufs=2)
                    nc.scalar.copy(dn[:], accs[hh][DH: DH + 1, :])
                    rc = msc.tile([1, NCH], f32, tag="rc", bufs=2)
                    nc.vector.reciprocal_approx_fast(rc[:], dn[:])
                    rcr = msc.tile([1, NCH], f16, tag="rcr", bufs=2)
                    nc.vector.tensor_copy(rcr[:], rc[:])  # round to f16
                    # broadcast partition 0 -> 64 partitions on idle GPSIMD
                    bsb = msc.tile([DH, NCH], f16, tag="bsb")
                    nc.gpsimd.partition_broadcast(bsb[:], rcr[:], channels=DH)
                    bsbs.append(bsb)

            def stage_b():
                nc.vector.tensor_mul(at[0:DH, p, dst_sl], accs[0][0:DH, :], bsbs[0][:])
                # DVE lanes can't shift partitions; bounce via DMA
                tmp = msc.tile([DH, NCH], f16, tag="tmp", bufs=2)
                nc.vector.tensor_mul(tmp[:], accs[1][0:DH, :], bsbs[1][:])
                nc.sync.dma_start(at[64:128, p, dst_sl], tmp[:])

            return stage_a, stage_b

        def make_scores(p, ic):
            def scores(j):
                # disjoint PE row groups (partitions 0-63 / 64-127): the two
                # K=64 matmuls execute concurrently
                sc = pss.tile([128, 2 * NCH], f32, tag="sc", name="sc")
                for hh in range(2):
                    nc.tensor.matmul(
                        sc[:, ts(hh, NCH)],
                        kt[hh * 64: hh * 64 + DH, p, ts(j, 128)],
                        qt[hh * 64: hh * 64 + DH, p, ts(ic, NCH)],
                        start=True, stop=True,
                    )
                return sc
            return scores

        def emit_exp(pe, sc, j, dve_js):
            if j in dve_js:
                pe0 = p0p.tile([128, 2 * NCH], f16, tag="pe0", name="pe0")
                nc.vector.tensor_scalar(
                    pe0[:].bitcast(i16), sc[:], EXP_ALPHA, EXP_BETA,
                    op0=Mult, op1=Add,
                )
                nc.vector._custom_dve(
                    polish_op, out=pe[:], in0=pe0[:], in1=pcc[:],
                    s0=maskc[:].bitcast(f32), s1=1.0, imm2=EXP_PB,
                )
            else:
                nc.scalar.activation(pe[:], sc[:], Exp, scale=0.125)

        seq = [(p, ic) for p in range(FT) for ic in range(T // NCH)]
        scores_of = {g: make_scores(*g) for g in seq}

        # filler schedule: extras[(gi, j)] = list of thunks
        extras = {}
        def add(gi, j, th):
            extras.setdefault((gi, j), []).append(th)

        for j in range(TT - 1):                     # g0: V proj just-in-time
            add(0, j, v_tile(j + 1))
        add(0, 2, qk_chunk(wks, bkc, kt, 0, 1))
        add(0, 5, qk_chunk(wks, bkc, kt, 0, 2))
        add(0, 9, qk_chunk(wks, bkc, kt, 0, 3))
        add(0, 13, qk_chunk(wqs, bqc, qt, 0, 1))
        add(1, 3, qk_chunk(wks, bkc, kt, 1, 0))
        add(1, 8, qk_chunk(wqs, bqc, qt, 0, 2))
        add(2, 3, qk_chunk(wks, bkc, kt, 1, 1))
        add(2, 8, qk_chunk(wqs, bqc, qt, 0, 3))
        add(3, 3, qk_chunk(wks, bkc, kt, 1, 2))
        add(3, 8, qk_chunk(wks, bkc, kt, 1, 3))
        add(3, 11, qk_chunk(wqs, bqc, qt, 1, 0))
        add(4, 3, qk_chunk(wqs, bqc, qt, 1, 1))
        add(4, 8, qk_chunk(wqs, bqc, qt, 1, 2))
        add(4, 12, qk_chunk(wqs, bqc, qt, 1, 3))
        for i in range(4):                          # out-proj, one ic behind
            add(5, 5 + 3 * i, outproj_tile(i))
            add(6, 5 + 3 * i, outproj_tile(4 + i))
            add(7, 5 + 3 * i, outproj_tile(8 + i))

        # ---- prologue: just enough projection for the first group ----
        v_tile(0)()
        qk_chunk(wks, bkc, kt, 0, 0)()
        qk_chunk(wqs, bqc, qt, 0, 0)()

        # ---- flat attention pipeline over all 8 groups ----
        def emit_pv(pe, accs, p, j):
            for hh in range(2):
                nc.tensor.matmul(
                    accs[hh][:, :],
                    vs[:, j, (2 * p + hh) * VW: (2 * p + hh + 1) * VW],
                    pe[:, ts(hh, NCH)],
                    start=(j == 0), stop=(j == TT - 1),
                )

        from collections import deque
        pending = deque()  # exp->PV emission lag
        sc_cur = scores_of[seq[0]](0)
        for gi, (p, ic) in enumerate(seq):
            dve_js = DVE_JS_H1 if gi < 4 else DVE_JS_H2
            acc0 = pso.tile([VW, NCH], f32, tag="acc0", name="acc0")
            acc1 = pso.tile([VW, NCH], f32, tag="acc1", name="acc1")
            accs = (acc0, acc1)
            for j in range(TT):
                pe = ptp.tile([128, 2 * NCH], f16, tag="pe", name="pe")
                emit_exp(pe, sc_cur, j, dve_js)
                pending.append((pe, accs, p, j))
                if j + 1 < TT:
                    sc_cur = scores_of[(p, ic)](j + 1)
                elif gi + 1 < len(seq):
                    sc_cur = scores_of[seq[gi + 1]](0)  # no exp-stream break
                tgt = PV_TGT.get(j, 3) if gi > 0 else 3
                while len(pending) > tgt:
                    emit_pv(*pending.popleft())
                for th in extras.get((gi, j), ()):
                    th()
            # normalization is staged at slot 2 of the next group: after the
            # PV(15) pop (data complete) and before the deferred PV(0) pop
            # (banks not yet recycled)
            stage_a, stage_b = make_norm(p, ic, accs)
            if gi + 1 < len(seq):
                add(gi + 1, 2, stage_a)
                add(gi + 1, 2, stage_b)
            else:
                while pending:
                    emit_pv(*pending.popleft())
                stage_a()
                stage_b()
        for t in range(12, 16):
            outproj_tile(t)()

    nc.finalize()  # Bacc.compile(): wait legalization, reg alloc, act tables
    return nc


def _get_program():
    global _prog
    if _prog is None:
        _prog = _build()
    return _prog


def kernel(x, W_qkv, b_qkv, W_out, b_out):
    global LAST_RESULT
    from concourse.bass_utils import run_bass_kernel_spmd

    x = np.asarray(x, np.float32)
    W_qkv = np.asarray(W_qkv, np.float32)
    b_qkv = np.asarray(b_qkv, np.float32)
    W_out = np.asarray(W_out, np.float32)
    b_out = np.asarray(b_out, np.float32)

    nc = _get_program()

    in_maps = []
    for c in range(NCORES):
        b, g = divmod(c, GROUPS)
        sl = slice(g * F, (g + 1) * F)
        # interleave Wv/bv with [zero-weight, bias=1] columns at h*65+64
        wv_g = W_qkv[:, 2 * D:3 * D][:, sl]
        bv_g = b_qkv[2 * D:3 * D][sl]
        wv_i = np.zeros((D, VF), np.float16)
        bv_i = np.zeros((1, VF), np.float16)
        for h in range(HPC):
            wv_i[:, h * VW: h * VW + DH] = wv_g[:, h * DH:(h + 1) * DH]
            bv_i[0, h * VW: h * VW + DH] = bv_g[h * DH:(h + 1) * DH]
            bv_i[0, h * VW + DH] = 1.0
        in_maps.append({
            "xT": np.ascontiguousarray(x[b].T.astype(np.float16)),
            "wq": np.ascontiguousarray(W_qkv[:, 0 * D:1 * D][:, sl]).astype(np.float16),
            "wk": np.ascontiguousarray(W_qkv[:, 1 * D:2 * D][:, sl]).astype(np.float16),
            "wv": wv_i,
            "bq": np.ascontiguousarray(b_qkv[0 * D:1 * D][sl][:, None]),
            "bk": np.ascontiguousarray(b_qkv[1 * D:2 * D][sl][:, None]),
            "bv": bv_i,
            "wo": np.ascontiguousarray(W_out[sl, :]).astype(np.float16),
        })

    kw = {}
    if os.environ.get("KERNEL_TRACE") == "1":
        kw["trace"] = True
    res = run_bass_kernel_spmd(nc, in_maps, core_ids=list(range(NCORES)), **kw)
    LAST_RESULT = res

    out = np.empty((B, T, D), np.float32)
    for b in range(B):
        acc = res.results[GROUPS * b]["out"].astype(np.float32)
        for g in range(1, GROUPS):
            acc = acc + res.results[GROUPS * b + g]["out"].astype(np.float32)
        out[b] = acc + b_out
    return out


# revision 27
# speedup vs baseline: 1.1311x; 1.0111x over previous
"""Multi-head attention (B=2, T=2048, D=1024, H=16, Dh=64) on 8 TRN2 NeuronCores.

Sharding: core c = 4*b + g  ->  batch b in {0,1}, head-group g in {0..3}
(4 heads per core: data parallel on batch, tensor parallel on heads).
Each core computes, for its batch element and its 4 heads:

  Q.T/K.T = Wq/k_shard.T @ x.T + b      [256, 2048]  (head-dim on partitions)
  V'      = x @ Wv_interleaved + b      [2048, 260]  ([V_h | 1] per head)
  per head pair (2p, 2p+1), per 512-wide i-chunk:
    S.T   = K_h Q_h.T                   (two K=64 matmuls on disjoint PE
                                         row groups -> run concurrently)
    P.T   = exp(S.T / 8)                (no max-subtraction: |S|/8 <~ 6)
    acc   = [V_h | 1].T @ P.T           [65, 512]  row 64 = softmax denom
    attnT = acc[:64] * (1/acc[64])
  partial = attnT.T @ Wout_shard        [2048, 1024]  (f16 out, host-summed)

The partial sum over the 4 head groups plus b_out is done on the host
("all-reduce after out_proj"), as is the batch unshard.

The softmax exp stream is split across two engines so neither serializes
the kernel: the scalar engine (ACT) computes true exp for most j-tiles;
for j in DVE_JS the vector engine computes a bit-trick exp2 -- one
tensor_scalar (x*a+b -> int16 bit pattern read as f16) plus one custom
DVE op that polishes the mantissa-linear error with a quadratic in
m = 1+frac (recovered exactly via bitwise and/or), max rel err ~5e-3.
PSUM evictions (V tiles, out-proj tiles, denominator rows) run on the
scalar engine; softmax normalization runs on the vector engine with the
reciprocal row broadcast across partitions by a 0-stride DMA.
"""

import os
import numpy as np

B, T, D = 2, 2048, 1024
H, DH = 16, 64
NCORES, GROUPS = 8, 4
HPC = H // GROUPS        # 4 heads per core
F = HPC * DH             # 256 features per core
FT = F // 128            # 2 feature tiles / head pairs
KTN = D // 128           # 8 contraction tiles
TT = T // 128            # 16 token tiles
NCH = 512                # matmul free-dim chunk
VW = DH + 1              # 65: V plus ones column
VF = HPC * VW            # 260: interleaved [V_h | 1] x 4 heads

# exp split: j-tiles per group computed on the DVE instead of ACT.
# Boundary js (13..15, 0..2) stay on ACT so the scores-PSUM recycle and
# the group-start PV never wait on the slower 2-instruction DVE path.
# Per-group counts follow each group's other engine load (qk bias adds on
# DVE in groups 0-4, out-proj evictions split ACT/DVE in groups 5-7).
DVE_JS_BY_GROUP = (
    frozenset({3, 5, 7, 9, 11}),
    frozenset({3, 5, 7, 9, 11}),
    frozenset({3, 5, 7, 9, 11}),
    frozenset({3, 5, 7, 9, 11}),
    frozenset({3, 5, 7, 9, 11}),
    frozenset({3, 4, 6, 8, 10, 12}),
    frozenset({3, 4, 6, 8, 10, 12}),
    frozenset({3, 5, 8, 10, 12}),
)
# PV matmuls trail exp emission by PV_TGT[j] tiles. The bulge at j=3..5
# suspends PV pops right after a group boundary: the previous group's
# normalization chain (4 engines deep) runs while the PE chews scores and
# extras instead of stalling on the freshly-recycled acc banks.
PV_TGT = {0: 3, 1: 3, 2: 3, 3: 6, 4: 6, 5: 6, 6: 5, 7: 4}
PV_LAG_MAX = 6

# f16 Schraudolph constants: i16 bits = round(s*ALPHA + BETA), then
# polish out = pe0 * ((m + PB) * m + PC), m = or(and(bits, MASK), 1.0).
# BETA includes the polish normalization (c0_adj = 2.0496...).
EXP_ALPHA = 184.6649652337873
EXP_BETA = 13261.182453842255
EXP_PB = -2.960537740957013
EXP_PC = 6.080904660347446
EXP_MASK = 0x007FE000

_prog = None
LAST_RESULT = None


def _register_exp_polish():
    """Register the EXP_POLISH_ANT custom DVE op (quadratic mantissa polish)."""
    import concourse.dve_ops as dom

    name = "EXP_POLISH_ANT"
    for op in dom.OPS:
        if op.name == name:
            return op
    from concourse.dve_spec import (
        Spec, Src0, C0, C1, C2, C3, Bin, AluOp, _spill_c3_to_src1, _has_src1, lower,
    )
    from concourse.dve_uop import DveOpSpec

    m = Bin(AluOp.BITWISE_OR, Bin(AluOp.BITWISE_AND, Src0, C0), C1)
    body = _spill_c3_to_src1(Src0 * ((m + C2) * m + C3))

    def ref(in0, in1, s0, s1, imm2):
        x = np.asarray(in0, np.float32)
        mask = np.asarray(s0, np.float32).view(np.uint32)
        orv = np.asarray(s1, np.float32).view(np.uint32)
        mm = ((x.view(np.uint32) & mask) | orv).view(np.float32)
        c = np.asarray(in1, np.float32)
        return (x * ((mm + np.float32(imm2)) * mm + c)).astype(np.float32)

    spec = Spec(body=body, reference=ref)
    row = max(dom._SUB_OPCODE_FOR_NAME.values()) + 1
    assert row < 0x20, "no free custom-DVE opcode row"
    dom._SUB_OPCODE_FOR_NAME[name] = row
    shas = {}
    for ver in ("v3", "v4"):
        try:
            shas[ver] = DveOpSpec(
                name=name, opcode=row, uops=lower(spec, ver=ver),
                rd1_en=_has_src1(spec),
            ).sha(ver)
        except Exception:
            if ver == "v3":
                raise  # TRN2 is v3; v4 is best-effort
    op = dom.DveOp(name, spec, subdim=False, uops_sha=shas)
    dom.OPS.append(op)
    dom.CUSTOM_DVE_SPECS[name] = spec
    return op


def _build():
    from contextlib import ExitStack

    import concourse.mybir as mybir
    import concourse.tile as tile
    from concourse import bacc
    from concourse.bass import ts

    f32 = mybir.dt.float32
    f16 = mybir.dt.float16
    i32 = mybir.dt.int32
    i16 = mybir.dt.int16
    Exp = mybir.ActivationFunctionType.Exp
    Mult = mybir.AluOpType.mult
    Add = mybir.AluOpType.add

    polish_op = _register_exp_polish()

    nc = bacc.Bacc()
    xT = nc.dram_tensor("xT", [D, T], f16, kind="ExternalInput")
    wq = nc.dram_tensor("wq", [D, F], f16, kind="ExternalInput")
    wk = nc.dram_tensor("wk", [D, F], f16, kind="ExternalInput")
    # wv/bv come pre-interleaved from the host: column h*65+64 is a zero
    # weight column whose bias is 1.0, producing the [V_h | 1] layout that
    # supplies the softmax-denominator row of the PV matmul for free.
    wv = nc.dram_tensor("wv", [D, VF], f16, kind="ExternalInput")
    bq = nc.dram_tensor("bq", [F, 1], f32, kind="ExternalInput")
    bk = nc.dram_tensor("bk", [F, 1], f32, kind="ExternalInput")
    bv = nc.dram_tensor("bv", [1, VF], f16, kind="ExternalInput")
    wo = nc.dram_tensor("wo", [F, D], f16, kind="ExternalInput")
    out = nc.dram_tensor("out", [T, D], f16, kind="ExternalOutput")

    from concourse.dve_ops import RECIP_APPROX_FAST_CONSTS, RECIPROCAL_APPROX_FAST

    with ExitStack() as ctx:
        tc = ctx.enter_context(tile.TileContext(nc))
        pers = ctx.enter_context(tc.tile_pool(name="pers", bufs=1))
        ptp = ctx.enter_context(tc.tile_pool(name="ptp", bufs=PV_LAG_MAX + 2))
        p0p = ctx.enter_context(tc.tile_pool(name="p0p", bufs=2))
        osb = ctx.enter_context(tc.tile_pool(name="osb", bufs=2))
        msc = ctx.enter_context(tc.tile_pool(name="msc", bufs=2))
        psq = ctx.enter_context(tc.tile_pool(name="psq", bufs=2, space="PSUM"))
        pss = ctx.enter_context(tc.tile_pool(name="pss", bufs=2, space="PSUM"))
        pso = ctx.enter_context(tc.tile_pool(name="pso", bufs=1, space="PSUM"))

        xt = pers.tile([128, KTN, T], f16, tag="xt")
        wqs = pers.tile([128, KTN, F], f16, tag="wqs")
        wks = pers.tile([128, KTN, F], f16, tag="wks")
        wvs = pers.tile([128, KTN, VF], f16, tag="wvs")
        bqc = pers.tile([128, FT, 1], f32, tag="bqc")
        bkc = pers.tile([128, FT, 1], f32, tag="bkc")
        bvr = pers.tile([1, VF], f16, tag="bvr")
        ones_f = pers.tile([1, 128], f32, tag="ones_f")
        ones16 = pers.tile([1, 128], f16, tag="ones16")
        maskc = pers.tile([128, 1], i32, tag="maskc")
        pcc = pers.tile([128, 1], f32, tag="pcc")
        wos = pers.tile([128, FT, D], f16, tag="wos")
        qt = pers.tile([128, FT, T], f16, tag="qt")
        kt = pers.tile([128, FT, T], f16, tag="kt")
        vs = pers.tile([128, TT, VF], f16, tag="vs")
        at = pers.tile([128, FT, T], f16, tag="at")

        # ISA memset can't target f16; memset f32 then copy-convert
        nc.vector.memset(ones_f[:], 1.0)
        nc.vector.tensor_copy(ones16[:], ones_f[:])
        nc.vector.memset(maskc[:], EXP_MASK)
        nc.vector.memset(pcc[:], EXP_PC)

        # ---- front loads: weights on the gpsimd queue, x on sync, both in
        # first-use order. xt comes in three 512-col waves so the first
        # scores group unblocks after ~2.5MB instead of the full 4MB.
        nc.gpsimd.dma_start(bvr[:], bv[:])
        for k in range(KTN):
            nc.gpsimd.dma_start(wvs[:, k, :], wv[ts(k, 128), :])
            nc.sync.dma_start(xt[:, k, 0:NCH], xT[ts(k, 128), 0:NCH])
        for k in range(KTN):
            nc.gpsimd.dma_start(wks[:, k, :], wk[ts(k, 128), :])
            nc.gpsimd.dma_start(wqs[:, k, :], wq[ts(k, 128), :])
            nc.sync.dma_start(xt[:, k, NCH:2 * NCH], xT[ts(k, 128), NCH:2 * NCH])
        for ft in range(FT):
            nc.gpsimd.dma_start(bkc[:, ft, :], bk[ts(ft, 128), :])
            nc.gpsimd.dma_start(bqc[:, ft, :], bq[ts(ft, 128), :])
        for k in range(KTN):
            nc.sync.dma_start(xt[:, k, 2 * NCH:T], xT[ts(k, 128), 2 * NCH:T])
        for ft in range(FT):
            nc.gpsimd.dma_start(wos[:, ft, :], wo[ts(ft, 128), :])

        # ---- PE warmup: ~4us of junk matmuls while the front DMAs land,
        # so the HAM clock gate reaches 8/8 before the real stream starts
        for _ in range(12):
            junk = psq.tile([128, VF], f32, tag="psq", name="junk")
            nc.tensor.matmul(junk[:], ones16[:, 0:128], bvr[:],
                             start=True, stop=True)

        # ---- deferred work units (emitted inside attention j-loops) ----
        def qk_chunk(wsb, bcol, dst, ft, c):
            def go():
                ps = psq.tile([128, NCH], f32, tag="psq", name="ps")
                for k in range(KTN):
                    nc.tensor.matmul(
                        ps[:],
                        wsb[:, k, ts(ft, 128)],
                        xt[:, k, ts(c, NCH)],
                        start=(k == 0), stop=(k == KTN - 1),
                    )
                nc.vector.tensor_scalar_add(
                    dst[:, ft, ts(c, NCH)], ps[:], bcol[:, ft, :]
                )
            return go

        def v_tile(t):
            def go():
                pv = psq.tile([128, VF], f32, tag="psq", name="pv")
                for k in range(KTN):
                    nc.tensor.matmul(
                        pv[:], xt[:, k, ts(t, 128)], wvs[:, k, :],
                        start=(k == 0), stop=False,
                    )
                # bias via ones-row (also writes the denominator 1.0 cols)
                nc.tensor.matmul(
                    pv[:], ones16[:, 0:128], bvr[:], start=False, stop=True
                )
                nc.scalar.copy(vs[:, t, :], pv[:])
            return go

        def outproj_tile(t, copy_engs=("scalar", "scalar")):
            def go():
                ob = osb.tile([128, D], f16, tag="ob", name="ob")
                # ft-outer so the two matmuls sharing lhsT are back-to-back
                pps = [psq.tile([128, NCH], f32, tag="psq", name="pp")
                       for _ in range(D // NCH)]
                for ft in range(FT):
                    for c in range(D // NCH):
                        nc.tensor.matmul(
                            pps[c][:],
                            at[:, ft, ts(t, 128)],
                            wos[:, ft, ts(c, NCH)],
                            start=(ft == 0), stop=(ft == FT - 1),
                        )
                for c in range(D // NCH):
                    if copy_engs[c] == "scalar":
                        nc.scalar.copy(ob[:, ts(c, NCH)], pps[c][:])
                    else:
                        nc.vector.tensor_copy(ob[:, ts(c, NCH)], pps[c][:])
                nc.sync.dma_start(out[ts(t, 128), :], ob[:])
            return go

        def make_norm(p, ic, accs):
            """Softmax normalization for group (p, ic): attnT = num/denom.
            Two stages emitted early in the NEXT group; the PV emission lag
            gives the cross-engine chain time to clear before the next
            group's first PV needs the acc banks back."""
            dst_sl = ts(ic, NCH)
            c = RECIP_APPROX_FAST_CONSTS
            bsbs = []

            def stage_a():
                for hh in range(2):
                    # denominator row 64 -> partition 0 (custom-DVE ops
                    # drop the partition base offset); copy on ACT
                    dn = msc.tile([1, NCH], f32, tag="dn", bufs=2)
                    nc.scalar.copy(dn[:], accs[hh][DH: DH + 1, :])
                    rc = msc.tile([1, NCH], f32, tag="rc", bufs=2)
                    nc.vector.reciprocal_approx_fast(rc[:], dn[:])
                    rcr = msc.tile([1, NCH], f16, tag="rcr", bufs=2)
                    nc.vector.tensor_copy(rcr[:], rc[:])  # round to f16
                    # broadcast partition 0 -> 64 partitions on idle GPSIMD
                    bsb = msc.tile([DH, NCH], f16, tag="bsb")
                    nc.gpsimd.partition_broadcast(bsb[:], rcr[:], channels=DH)
                    bsbs.append(bsb)

            def stage_b():
                nc.vector.tensor_mul(at[0:DH, p, dst_sl], accs[0][0:DH, :], bsbs[0][:])
                # DVE lanes can't shift partitions; bounce via DMA
                tmp = msc.tile([DH, NCH], f16, tag="tmp", bufs=2)
                nc.vector.tensor_mul(tmp[:], accs[1][0:DH, :], bsbs[1][:])
                nc.sync.dma_start(at[64:128, p, dst_sl], tmp[:])

            return stage_a, stage_b

        def make_scores(p, ic):
            def scores(j):
                # disjoint PE row groups (partitions 0-63 / 64-127): the two
                # K=64 matmuls execute concurrently
                sc = pss.tile([128, 2 * NCH], f32, tag="sc", name="sc")
                for hh in range(2):
                    nc.tensor.matmul(
                        sc[:, ts(hh, NCH)],
                        kt[hh * 64: hh * 64 + DH, p, ts(j, 128)],
                        qt[hh * 64: hh * 64 + DH, p, ts(ic, NCH)],
                        start=True, stop=True,
                    )
                return sc
            return scores

        def emit_exp(pe, sc, j, dve_js):
            if j in dve_js:
                pe0 = p0p.tile([128, 2 * NCH], f16, tag="pe0", name="pe0")
                nc.vector.tensor_scalar(
                    pe0[:].bitcast(i16), sc[:], EXP_ALPHA, EXP_BETA,
                    op0=Mult, op1=Add,
                )
                nc.vector._custom_dve(
                    polish_op, out=pe[:], in0=pe0[:], in1=pcc[:],
                    s0=maskc[:].bitcast(f32), s1=1.0, imm2=EXP_PB,
                )
            else:
                nc.scalar.activation(pe[:], sc[:], Exp, scale=0.125)

        seq = [(p, ic) for p in range(FT) for ic in range(T // NCH)]
        scores_of = {g: make_scores(*g) for g in seq}

        # filler schedule: extras[(gi, j)] = list of thunks
        extras = {}
        def add(gi, j, th):
            extras.setdefault((gi, j), []).append(th)

        for j in range(TT - 1):                     # g0: V proj just-in-time
            add(0, j, v_tile(j + 1))
        add(0, 2, qk_chunk(wks, bkc, kt, 0, 1))
        add(0, 5, qk_chunk(wks, bkc, kt, 0, 2))
        add(0, 9, qk_chunk(wks, bkc, kt, 0, 3))
        add(0, 13, qk_chunk(wqs, bqc, qt, 0, 1))
        add(1, 3, qk_chunk(wks, bkc, kt, 1, 0))
        add(1, 8, qk_chunk(wqs, bqc, qt, 0, 2))
        add(2, 3, qk_chunk(wks, bkc, kt, 1, 1))
        add(2, 8, qk_chunk(wqs, bqc, qt, 0, 3))
        add(3, 3, qk_chunk(wks, bkc, kt, 1, 2))
        add(3, 8, qk_chunk(wks, bkc, kt, 1, 3))
        add(3, 11, qk_chunk(wqs, bqc, qt, 1, 0))
        add(4, 3, qk_chunk(wqs, bqc, qt, 1, 1))
        add(4, 8, qk_chunk(wqs, bqc, qt, 1, 2))
        add(4, 12, qk_chunk(wqs, bqc, qt, 1, 3))
        for i in range(4):                          # out-proj, one ic behind
            add(5, 5 + 3 * i, outproj_tile(i))
            add(6, 5 + 3 * i, outproj_tile(4 + i))
            add(7, 5 + 3 * i, outproj_tile(8 + i, ("scalar", "vector")))

        # ---- prologue: just enough projection for the first group ----
        v_tile(0)()
        qk_chunk(wks, bkc, kt, 0, 0)()
        qk_chunk(wqs, bqc, qt, 0, 0)()

        # ---- flat attention pipeline over all 8 groups ----
        def emit_pv(pe, accs, p, j):
            for hh in range(2):
                nc.tensor.matmul(
                    accs[hh][:, :],
                    vs[:, j, (2 * p + hh) * VW: (2 * p + hh + 1) * VW],
                    pe[:, ts(hh, NCH)],
                    start=(j == 0), stop=(j == TT - 1),
                )

        from collections import deque
        pending = deque()  # exp->PV emission lag
        sc_cur = scores_of[seq[0]](0)
        for gi, (p, ic) in enumerate(seq):
            dve_js = DVE_JS_BY_GROUP[gi]
            acc0 = pso.tile([VW, NCH], f32, tag="acc0", name="acc0")
            acc1 = pso.tile([VW, NCH], f32, tag="acc1", name="acc1")
            accs = (acc0, acc1)
            for j in range(TT):
                pe = ptp.tile([128, 2 * NCH], f16, tag="pe", name="pe")
                emit_exp(pe, sc_cur, j, dve_js)
                pending.append((pe, accs, p, j))
                if j + 1 < TT:
                    sc_cur = scores_of[(p, ic)](j + 1)
                elif gi + 1 < len(seq):
                    sc_cur = scores_of[seq[gi + 1]](0)  # no exp-stream break
                tgt = PV_TGT.get(j, 3) if gi > 0 else 3
                while len(pending) > tgt:
                    emit_pv(*pending.popleft())
                for th in extras.get((gi, j), ()):
                    th()
            # normalization is staged at slot 2 of the next group: after the
            # PV(15) pop (data complete) and before the deferred PV(0) pop
            # (banks not yet recycled)
            stage_a, stage_b = make_norm(p, ic, accs)
            if gi + 1 < len(seq):
                add(gi + 1, 2, stage_a)
                add(gi + 1, 2, stage_b)
            else:
                while pending:
                    emit_pv(*pending.popleft())
                stage_a()
                stage_b()
        for t in range(12, 16):
            outproj_tile(t, ("scalar", "vector"))()

    nc.finalize()  # Bacc.compile(): wait legalization, reg alloc, act tables
    return nc


def _get_program():
    global _prog
    if _prog is None:
        _prog = _build()
    return _prog


def kernel(x, W_qkv, b_qkv, W_out, b_out):
    global LAST_RESULT
    from concourse.bass_utils import run_bass_kernel_spmd

    x = np.asarray(x, np.float32)
    W_qkv = np.asarray(W_qkv, np.float32)
    b_qkv = np.asarray(b_qkv, np.float32)
    W_out = np.asarray(W_out, np.float32)
    b_out = np.asarray(b_out, np.float32)

    nc = _get_program()

    in_maps = []
    for c in range(NCORES):
        b, g = divmod(c, GROUPS)
        sl = slice(g * F, (g + 1) * F)
        # interleave Wv/bv with [zero-weight, bias=1] columns at h*65+64
        wv_g = W_qkv[:, 2 * D:3 * D][:, sl]
        bv_g = b_qkv[2 * D:3 * D][sl]
        wv_i = np.zeros((D, VF), np.float16)
        bv_i = np.zeros((1, VF), np.float16)
        for h in range(HPC):
            wv_i[:, h * VW: h * VW + DH] = wv_g[:, h * DH:(h + 1) * DH]
            bv_i[0, h * VW: h * VW + DH] = bv_g[h * DH:(h + 1) * DH]
            bv_i[0, h * VW + DH] = 1.0
        in_maps.append({
            "xT": np.ascontiguousarray(x[b].T.astype(np.float16)),
            "wq": np.ascontiguousarray(W_qkv[:, 0 * D:1 * D][:, sl]).astype(np.float16),
            "wk": np.ascontiguousarray(W_qkv[:, 1 * D:2 * D][:, sl]).astype(np.float16),
            "wv": wv_i,
            "bq": np.ascontiguousarray(b_qkv[0 * D:1 * D][sl][:, None]),
            "bk": np.ascontiguousarray(b_qkv[1 * D:2 * D][sl][:, None]),
            "bv": bv_i,
            "wo": np.ascontiguousarray(W_out[sl, :]).astype(np.float16),
        })

    kw = {}
    if os.environ.get("KERNEL_TRACE") == "1":
        kw["trace"] = True
    res = run_bass_kernel_spmd(nc, in_maps, core_ids=list(range(NCORES)), **kw)
    LAST_RESULT = res

    out = np.empty((B, T, D), np.float32)
    for b in range(B):
        acc = res.results[GROUPS * b]["out"].astype(np.float32)
        for g in range(1, GROUPS):
            acc = acc + res.results[GROUPS * b + g]["out"].astype(np.float32)
        out[b] = acc + b_out
    return out
